# revision 1
# baseline (speedup 1.0000x reference)
"""Trainium2 Bass kernel for nn_Conv1dBlock (LIF spikes -> Conv1d(k=5, same) -> GroupNorm).

Contract: kernel(**inputs) takes FULL inputs (x [4,64,256,512] f32, conv_w
[256,256,5], conv_b/gamma/beta [256]) and returns the FULL [4,64,256,512] f32
output. Internally shards data-parallel over B across 8 NeuronCores.

Per-core algorithm (B_loc = 8):
  - LIF (VectorE, fp32, op-order bit-matching the reference):
      d = x - v; v = 0.5*d + v; s = (v >= 0.5) -> bf16; v = (v < 0.5) * v
  - Conv1d as 5 shifted matmuls per (ci_tile, co_tile) accumulated in PSUM.
    Weights split W = hi + lo (both bf16); spikes are exactly representable
    in bf16, so the pair of matmuls reproduces fp32-level accuracy (~2.5e-6).
  - GroupNorm without ever adding conv bias to the [128,512] data:
      r = sum_l y, q = sum_l y^2 (ScalarE activation accum_out)
      t1 = r + 512 b ; t2 = q + 2 b r + 512 b^2   (per-channel, tiny)
      group sums via ones-block matmul; mu/var/rsqrt on 4-8 lanes;
      broadcast back via ones matmul; out = y*A + B on ScalarE where
      A = kappa*gamma, B = (b - mu)*A + beta.
"""

import numpy as np
import ml_dtypes

T, B_FULL, C, L, K = 4, 64, 256, 512, 5
N_CORES = 8
B_LOC = B_FULL // N_CORES
G = 8            # groups
GPC = C // G     # 32 channels per group
CT = 2           # 128-channel tiles
EPS = 1e-5
NORM_N = GPC * L  # 32*512 elements per group

_COMPILED = {}


def _build_program():
    import concourse.bass as bass
    import concourse.tile as tile
    from concourse import bacc, mybir

    f32 = mybir.dt.float32
    bf16 = mybir.dt.bfloat16
    Alu = mybir.AluOpType
    Act = mybir.ActivationFunctionType

    nc = bacc.Bacc(
        "TRN2",
        target_bir_lowering=False,
        debug=False,
        num_devices=N_CORES,
    )

    x_d = nc.dram_tensor("x", [T, B_LOC, C, L], f32, kind="ExternalInput").ap()
    # [ci, prec(hi/lo), k, ci_t, co_t, co]
    w_d = nc.dram_tensor("w", [128, 2, K, 2, CT, 128], bf16, kind="ExternalInput").ap()
    # [co, field, co_t]; fields: b, gamma, beta, 512b, 2b, 512b^2
    chan_d = nc.dram_tensor("chan", [128, 6, CT], f32, kind="ExternalInput").ap()
    onesg_d = nc.dram_tensor("onesg", [128, 4], f32, kind="ExternalInput").ap()
    onesb_d = nc.dram_tensor("onesb", [128, 128], f32, kind="ExternalInput").ap()
    y_d = nc.dram_tensor("y", [T, B_LOC, C, L], f32, kind="ExternalOutput").ap()

    with tile.TileContext(nc) as tc:
        with (
            tc.tile_pool(name="singles", bufs=1) as singles,
            tc.tile_pool(name="xp", bufs=8) as xp,
            tc.tile_pool(name="sp", bufs=4) as sp,
            tc.tile_pool(name="dp", bufs=2) as dp,
            tc.tile_pool(name="ysb", bufs=6) as ysb,
            tc.tile_pool(name="smallsb", bufs=4) as smallsb,
            tc.tile_pool(name="ypsum", bufs=6, space="PSUM") as ypsum,
            tc.tile_pool(name="spsum", bufs=2, space="PSUM") as spsum,
        ):
            # ---- constants / parameters in SBUF ----
            w_s = singles.tile([128, 2, K, 2, CT, 128], bf16)
            nc.sync.dma_start(out=w_s[:], in_=w_d[:])
            chan = singles.tile([128, 6, CT], f32)
            nc.sync.dma_start(out=chan[:], in_=chan_d[:])
            onesg = singles.tile([128, 4], f32)
            nc.sync.dma_start(out=onesg[:], in_=onesg_d[:])
            onesb = singles.tile([128, 128], f32)
            nc.sync.dma_start(out=onesb[:], in_=onesb_d[:])
            eps_t = singles.tile([128, 1], f32)
            nc.vector.memset(eps_t[:], EPS)

            # persistent LIF membrane state per local batch element
            v_tiles = []
            for b in range(B_LOC):
                vt = singles.tile([128, 2, L], f32, tag=f"v{b}")
                nc.vector.memset(vt[:], 0.0)
                v_tiles.append(vt)

            def chan_col(field, ct):
                return chan[:, field, ct : ct + 1]

            # tap -> (rhs_lo, rhs_hi, out_lo, out_hi) column ranges
            tap_slices = []
            for k in range(K):
                d = k - 2
                if d >= 0:
                    tap_slices.append((d, L, 0, L - d))
                else:
                    tap_slices.append((0, L + d, -d, L))

            for t in range(T):
                for b in range(B_LOC):
                    xt = xp.tile([128, 2, L], f32)
                    nc.sync.dma_start(
                        out=xt[:],
                        in_=x_d[t, b].rearrange("(i p) l -> p i l", p=128),
                    )
                    v = v_tiles[b]
                    st = sp.tile([128, 2, L], bf16)
                    d_t = dp.tile([128, 2, L], f32)
                    # LIF step (all [128, 2, 512] views)
                    nc.vector.tensor_sub(out=d_t[:], in0=xt[:], in1=v[:])
                    nc.vector.scalar_tensor_tensor(
                        out=v[:], in0=d_t[:], scalar=0.5, in1=v[:],
                        op0=Alu.mult, op1=Alu.add,
                    )
                    nc.vector.tensor_scalar(
                        out=st[:], in0=v[:], scalar1=0.5, scalar2=None,
                        op0=Alu.is_ge,
                    )
                    nc.vector.scalar_tensor_tensor(
                        out=v[:], in0=v[:], scalar=0.5, in1=v[:],
                        op0=Alu.is_lt, op1=Alu.mult,
                    )

                    # conv + stats per co-tile
                    small_ps = spsum.tile([128, 8], f32)  # gsum cols 0:4, bc ct at 4+2ct
                    stats_tiles = []
                    y_sbs = []
                    for ct in range(CT):
                        yp = ypsum.tile([128, L], f32)
                        # matmul order: full-width center tap first (start=True)
                        mm_list = []
                        for prec in range(2):
                            for ci_t in range(2):
                                for k in range(K):
                                    mm_list.append((prec, ci_t, k))
                        mm_list.remove((0, 0, 2))
                        mm_list.insert(0, (0, 0, 2))
                        n_mm = len(mm_list)
                        for i, (prec, ci_t, k) in enumerate(mm_list):
                            rl, rh, ol, oh = tap_slices[k]
                            nc.tensor.matmul(
                                yp[:, ol:oh],
                                w_s[:, prec, k, ci_t, ct, :],
                                st[:, ci_t, rl:rh],
                                start=(i == 0),
                                stop=(i == n_mm - 1),
                                skip_group_check=True,
                            )
                        y_sb = ysb.tile([128, L], f32)
                        stats = smallsb.tile([128, 4], f32)
                        # r = sum_l y  (and copy PSUM -> SBUF)
                        nc.scalar.activation(
                            out=y_sb[:], in_=yp[:], func=Act.Copy,
                            accum_out=stats[:, 0:1],
                        )
                        # q = sum_l y^2 (squares PSUM in place; last PSUM use)
                        nc.scalar.activation(
                            out=yp[:], in_=yp[:], func=Act.Square,
                            accum_out=stats[:, 1:2],
                        )
                        # t1 = r + 512 b ; t2 = (r * 2b + q) + 512 b^2
                        nc.vector.tensor_add(
                            out=stats[:, 2:3], in0=stats[:, 0:1], in1=chan_col(3, ct)
                        )
                        nc.vector.scalar_tensor_tensor(
                            out=stats[:, 3:4], in0=stats[:, 0:1],
                            scalar=chan_col(4, ct), in1=stats[:, 1:2],
                            op0=Alu.mult, op1=Alu.add,
                        )
                        nc.vector.tensor_add(
                            out=stats[:, 3:4], in0=stats[:, 3:4], in1=chan_col(5, ct)
                        )
                        # group sums for this co-tile's 4 groups
                        nc.tensor.matmul(
                            small_ps[0:4, ct * 2 : ct * 2 + 2],
                            onesg[:],
                            stats[:, 2:4],
                            start=True,
                            stop=True,
                        )
                        stats_tiles.append(stats)
                        y_sbs.append(y_sb)

                    # mu/kappa for all 8 groups (4 partitions x 2 co-tiles)
                    mk = smallsb.tile([128, 2, 2], f32)  # [grp, ct, (mu,kappa)]
                    nc.vector.memset(mk[:], 0.0)
                    m2 = smallsb.tile([4, 2], f32)
                    vr = smallsb.tile([4, 2], f32)
                    gs = small_ps[0:4, 0:4].rearrange("p (c s) -> p s c", s=2)
                    mu_v = mk[0:4, :, 0]
                    nc.vector.tensor_scalar(
                        out=mu_v, in0=gs[:, 0, :], scalar1=1.0 / NORM_N,
                        scalar2=None, op0=Alu.mult,
                    )
                    nc.vector.tensor_mul(out=m2[:], in0=mu_v, in1=mu_v)
                    nc.vector.scalar_tensor_tensor(
                        out=vr[:], in0=gs[:, 1, :], scalar=1.0 / NORM_N, in1=m2[:],
                        op0=Alu.mult, op1=Alu.subtract,
                    )
                    nc.scalar.activation(
                        out=vr[:], in_=vr[:], func=Act.Sqrt, bias=eps_t[0:4],
                    )
                    nc.vector.reciprocal(out=mk[0:4, :, 1], in_=vr[:])

                    for ct in range(CT):
                        bc = small_ps[:, 4 + 2 * ct : 6 + 2 * ct]
                        nc.tensor.matmul(
                            bc, onesb[:], mk[:, ct, :], start=True, stop=True,
                        )
                        ab = smallsb.tile([128, 3], f32)
                        # A = kappa * gamma
                        nc.vector.tensor_mul(
                            out=ab[:, 0:1], in0=bc[:, 1:2], in1=chan_col(1, ct)
                        )
                        # B = (b - mu) * A + beta
                        nc.vector.tensor_sub(
                            out=ab[:, 2:3], in0=chan_col(0, ct), in1=bc[:, 0:1]
                        )
                        nc.vector.scalar_tensor_tensor(
                            out=ab[:, 1:2], in0=ab[:, 2:3], scalar=ab[:, 0:1],
                            in1=chan_col(2, ct), op0=Alu.mult, op1=Alu.add,
                        )
                        # out = y * A + B  (ScalarE affine, in place on y_sb)
                        y_sb = y_sbs[ct]
                        nc.scalar.activation(
                            out=y_sb[:], in_=y_sb[:], func=Act.Identity,
                            bias=ab[:, 1:2], scale=ab[:, 0:1],
                        )
                        nc.sync.dma_start(
                            out=y_d[t, b].rearrange("(i p) l -> p i l", p=128)[:, ct, :],
                            in_=y_sb[:],
                        )

    nc.compile()
    return nc


def _prep_host_inputs(x, conv_w, conv_b, gamma, beta):
    x = np.asarray(x, dtype=np.float32)
    conv_w = np.asarray(conv_w, dtype=np.float32)
    conv_b = np.asarray(conv_b, dtype=np.float32)
    gamma = np.asarray(gamma, dtype=np.float32)
    beta = np.asarray(beta, dtype=np.float32)

    # lhsT tiles: [ci, prec, k, ci_t, co_t, co]
    Wt = conv_w.transpose(1, 0, 2)                      # [ci_g, co_g, k]
    W6 = Wt.reshape(2, 128, CT, 128, K)                 # [ci_t, ci, co_t, co, k]
    whi32 = W6.astype(ml_dtypes.bfloat16).astype(np.float32)
    wlo = (W6 - whi32).astype(ml_dtypes.bfloat16)
    whi = W6.astype(ml_dtypes.bfloat16)
    w_host = np.stack(
        [whi.transpose(1, 4, 0, 2, 3), wlo.transpose(1, 4, 0, 2, 3)], axis=1
    )                                                   # [ci, prec, k, ci_t, co_t, co]
    w_host = np.ascontiguousarray(w_host)

    b = conv_b
    fields = np.stack(
        [b, gamma, beta, np.float32(L) * b, np.float32(2.0) * b,
         np.float32(L) * b * b]
    )                                                   # [6, 256]
    chan = np.ascontiguousarray(fields.reshape(6, CT, 128).transpose(2, 0, 1))

    onesg = np.zeros((128, 4), np.float32)
    for ci in range(128):
        onesg[ci, ci // GPC] = 1.0
    onesb = np.zeros((128, 128), np.float32)
    for co in range(128):
        onesb[co // GPC, co] = 1.0

    shards = []
    for i in range(N_CORES):
        shards.append(
            {
                "x": np.ascontiguousarray(x[:, i * B_LOC : (i + 1) * B_LOC]),
                "w": w_host,
                "chan": chan,
                "onesg": onesg,
                "onesb": onesb,
            }
        )
    return shards


def kernel(x, conv_w, conv_b, gamma, beta, _trace=False):
    from concourse.bass_utils import run_bass_kernel_spmd

    if "nc" not in _COMPILED:
        _COMPILED["nc"] = _build_program()
    nc = _COMPILED["nc"]

    in_maps = _prep_host_inputs(x, conv_w, conv_b, gamma, beta)
    res = run_bass_kernel_spmd(
        nc, in_maps, list(range(N_CORES)), trace=_trace
    )
    out = np.concatenate([r["y"] for r in res.results], axis=1)
    _COMPILED["last_result"] = res
    return out


# revision 2
# speedup vs baseline: 1.0556x; 1.0556x over previous
"""Trainium2 Bass kernel for nn_Conv1dBlock (LIF spikes -> Conv1d(k=5, same) -> GroupNorm).

Contract: kernel(**inputs) takes FULL inputs (x [4,64,256,512] f32, conv_w
[256,256,5], conv_b/gamma/beta [256]) and returns the FULL [4,64,256,512] f32
output. Internally shards data-parallel over B across 8 NeuronCores.

Per-core algorithm (B_loc = 8):
  - LIF (VectorE, fp32, op-order bit-matching the reference):
      d = x - v; v = 0.5*d + v; s = (v >= 0.5) -> bf16; v = (v < 0.5) * v
  - Conv1d as 5 shifted matmuls per (ci_tile, co_tile) accumulated in PSUM.
    Weights split W = hi + lo (both bf16); spikes are exactly representable
    in bf16, so the pair of matmuls reproduces fp32-level accuracy (~2.5e-6).
  - GroupNorm without ever adding conv bias to the [128,512] data:
      r = sum_l y, q = sum_l y^2 (ScalarE activation accum_out)
      t1 = r + 512 b ; t2 = q + 2 b r + 512 b^2   (per-channel, tiny)
      group sums via ones-block matmul; mu/var/rsqrt on 4-8 lanes;
      broadcast back via ones matmul; out = y*A + B on ScalarE where
      A = kappa*gamma, B = (b - mu)*A + beta.
"""

import numpy as np
import ml_dtypes

T, B_FULL, C, L, K = 4, 64, 256, 512, 5
N_CORES = 8
B_LOC = B_FULL // N_CORES
G = 8            # groups
GPC = C // G     # 32 channels per group
CT = 2           # 128-channel tiles
EPS = 1e-5
NORM_N = GPC * L  # 32*512 elements per group

_COMPILED = {}


def _build_program():
    import concourse.bass as bass
    import concourse.tile as tile
    from concourse import bacc, mybir

    f32 = mybir.dt.float32
    bf16 = mybir.dt.bfloat16
    Alu = mybir.AluOpType
    Act = mybir.ActivationFunctionType

    nc = bacc.Bacc(
        "TRN2",
        target_bir_lowering=False,
        debug=False,
        num_devices=N_CORES,
    )

    x_d = nc.dram_tensor("x", [T, B_LOC, C, L], f32, kind="ExternalInput").ap()
    # [ci, prec(hi/lo), k, ci_t, co_t, co]
    w_d = nc.dram_tensor("w", [128, 2, K, 2, CT, 128], bf16, kind="ExternalInput").ap()
    # [co, field, co_t]; fields: b, gamma, beta, 512b, 2b, 512b^2
    chan_d = nc.dram_tensor("chan", [128, 6, CT], f32, kind="ExternalInput").ap()
    onesg_d = nc.dram_tensor("onesg", [128, 4], f32, kind="ExternalInput").ap()
    onesb_d = nc.dram_tensor("onesb", [128, 128], f32, kind="ExternalInput").ap()
    y_d = nc.dram_tensor("y", [T, B_LOC, C, L], f32, kind="ExternalOutput").ap()

    with tile.TileContext(nc) as tc:
        with (
            tc.tile_pool(name="singles", bufs=1) as singles,
            tc.tile_pool(name="xp", bufs=8) as xp,
            tc.tile_pool(name="sp", bufs=4) as sp,
            tc.tile_pool(name="dp", bufs=2) as dp,
            tc.tile_pool(name="ysb", bufs=6) as ysb,
            tc.tile_pool(name="smallsb", bufs=4) as smallsb,
            tc.tile_pool(name="ypsum", bufs=6, space="PSUM") as ypsum,
            tc.tile_pool(name="spsum", bufs=2, space="PSUM") as spsum,
        ):
            # ---- constants / parameters in SBUF ----
            w_s = singles.tile([128, 2, K, 2, CT, 128], bf16)
            nc.sync.dma_start(out=w_s[:], in_=w_d[:])
            chan = singles.tile([128, 6, CT], f32)
            nc.sync.dma_start(out=chan[:], in_=chan_d[:])
            onesg = singles.tile([128, 4], f32)
            nc.sync.dma_start(out=onesg[:], in_=onesg_d[:])
            onesb = singles.tile([128, 128], f32)
            nc.sync.dma_start(out=onesb[:], in_=onesb_d[:])
            eps_t = singles.tile([128, 1], f32)
            nc.vector.memset(eps_t[:], EPS)

            # persistent LIF membrane state per local batch element
            v_tiles = []
            for b in range(B_LOC):
                vt = singles.tile([128, 2, L], f32, tag=f"v{b}")
                nc.vector.memset(vt[:], 0.0)
                v_tiles.append(vt)

            def chan_col(field, ct):
                return chan[:, field, ct : ct + 1]

            # tap -> (rhs_lo, rhs_hi, out_lo, out_hi) column ranges
            tap_slices = []
            for k in range(K):
                d = k - 2
                if d >= 0:
                    tap_slices.append((d, L, 0, L - d))
                else:
                    tap_slices.append((0, L + d, -d, L))

            def emit_tail(pend):
                """Group-sum + broadcast matmuls, A/B, normalize, store for a
                sample whose conv + stats are already emitted. Deferred one
                sample so these PE instructions never stall TensorE."""
                t, b, small_ps, stats_tiles, y_sbs = pend
                for ct in range(CT):
                    stats = stats_tiles[ct]
                    nc.tensor.matmul(
                        small_ps[0:4, ct * 2 : ct * 2 + 2],
                        onesg[:],
                        stats[:, 2:4],
                        start=True,
                        stop=True,
                    )
                # mu/kappa for all 8 groups (4 partitions x 2 co-tiles)
                mk = smallsb.tile([128, 2, 2], f32)  # [grp, ct, (mu,kappa)]
                nc.vector.memset(mk[:], 0.0)
                m2 = smallsb.tile([4, 2], f32)
                vr = smallsb.tile([4, 2], f32)
                gs = small_ps[0:4, 0:4].rearrange("p (c s) -> p s c", s=2)
                mu_v = mk[0:4, :, 0]
                nc.vector.tensor_scalar(
                    out=mu_v, in0=gs[:, 0, :], scalar1=1.0 / NORM_N,
                    scalar2=None, op0=Alu.mult,
                )
                nc.vector.tensor_mul(out=m2[:], in0=mu_v, in1=mu_v)
                nc.vector.scalar_tensor_tensor(
                    out=vr[:], in0=gs[:, 1, :], scalar=1.0 / NORM_N, in1=m2[:],
                    op0=Alu.mult, op1=Alu.subtract,
                )
                nc.scalar.activation(
                    out=vr[:], in_=vr[:], func=Act.Sqrt, bias=eps_t[0:4],
                )
                nc.vector.reciprocal(out=mk[0:4, :, 1], in_=vr[:])

                for ct in range(CT):
                    bc = small_ps[:, 4 + 2 * ct : 6 + 2 * ct]
                    nc.tensor.matmul(
                        bc, onesb[:], mk[:, ct, :], start=True, stop=True,
                    )
                    ab = smallsb.tile([128, 3], f32)
                    # A = kappa * gamma
                    nc.vector.tensor_mul(
                        out=ab[:, 0:1], in0=bc[:, 1:2], in1=chan_col(1, ct)
                    )
                    # B = (b - mu) * A + beta
                    nc.vector.tensor_sub(
                        out=ab[:, 2:3], in0=chan_col(0, ct), in1=bc[:, 0:1]
                    )
                    nc.vector.scalar_tensor_tensor(
                        out=ab[:, 1:2], in0=ab[:, 2:3], scalar=ab[:, 0:1],
                        in1=chan_col(2, ct), op0=Alu.mult, op1=Alu.add,
                    )
                    # out = y * A + B  (ScalarE affine, in place on y_sb)
                    y_sb = y_sbs[ct]
                    nc.scalar.activation(
                        out=y_sb[:], in_=y_sb[:], func=Act.Identity,
                        bias=ab[:, 1:2], scale=ab[:, 0:1],
                    )
                    nc.sync.dma_start(
                        out=y_d[t, b].rearrange("(i p) l -> p i l", p=128)[:, ct, :],
                        in_=y_sb[:],
                    )

            pending = None
            for t in range(T):
                for b in range(B_LOC):
                    xt = xp.tile([128, 2, L], f32)
                    nc.sync.dma_start(
                        out=xt[:],
                        in_=x_d[t, b].rearrange("(i p) l -> p i l", p=128),
                    )
                    v = v_tiles[b]
                    st = sp.tile([128, 2, L], bf16)
                    d_t = dp.tile([128, 2, L], f32)
                    # LIF step (all [128, 2, 512] views)
                    nc.vector.tensor_sub(out=d_t[:], in0=xt[:], in1=v[:])
                    nc.vector.scalar_tensor_tensor(
                        out=v[:], in0=d_t[:], scalar=0.5, in1=v[:],
                        op0=Alu.mult, op1=Alu.add,
                    )
                    nc.vector.tensor_scalar(
                        out=st[:], in0=v[:], scalar1=0.5, scalar2=None,
                        op0=Alu.is_ge,
                    )
                    nc.vector.scalar_tensor_tensor(
                        out=v[:], in0=v[:], scalar=0.5, in1=v[:],
                        op0=Alu.is_lt, op1=Alu.mult,
                    )

                    # conv + stats per co-tile
                    small_ps = spsum.tile([128, 8], f32)  # gsum cols 0:4, bc ct at 4+2ct
                    stats_tiles = []
                    y_sbs = []
                    for ct in range(CT):
                        yp = ypsum.tile([128, L], f32)
                        # matmul order: full-width center tap first (start=True)
                        mm_list = []
                        for prec in range(2):
                            for ci_t in range(2):
                                for k in range(K):
                                    mm_list.append((prec, ci_t, k))
                        mm_list.remove((0, 0, 2))
                        mm_list.insert(0, (0, 0, 2))
                        n_mm = len(mm_list)
                        for i, (prec, ci_t, k) in enumerate(mm_list):
                            rl, rh, ol, oh = tap_slices[k]
                            nc.tensor.matmul(
                                yp[:, ol:oh],
                                w_s[:, prec, k, ci_t, ct, :],
                                st[:, ci_t, rl:rh],
                                start=(i == 0),
                                stop=(i == n_mm - 1),
                                skip_group_check=True,
                            )
                        y_sb = ysb.tile([128, L], f32)
                        stats = smallsb.tile([128, 4], f32)
                        # r = sum_l y  (and copy PSUM -> SBUF)
                        nc.scalar.activation(
                            out=y_sb[:], in_=yp[:], func=Act.Copy,
                            accum_out=stats[:, 0:1],
                        )
                        # q = sum_l y^2 (squares PSUM in place; last PSUM use)
                        nc.scalar.activation(
                            out=yp[:], in_=yp[:], func=Act.Square,
                            accum_out=stats[:, 1:2],
                        )
                        # t1 = r + 512 b ; t2 = (r * 2b + q) + 512 b^2
                        nc.vector.tensor_add(
                            out=stats[:, 2:3], in0=stats[:, 0:1], in1=chan_col(3, ct)
                        )
                        nc.vector.scalar_tensor_tensor(
                            out=stats[:, 3:4], in0=stats[:, 0:1],
                            scalar=chan_col(4, ct), in1=stats[:, 1:2],
                            op0=Alu.mult, op1=Alu.add,
                        )
                        nc.vector.tensor_add(
                            out=stats[:, 3:4], in0=stats[:, 3:4], in1=chan_col(5, ct)
                        )
                        stats_tiles.append(stats)
                        y_sbs.append(y_sb)

                    if pending is not None:
                        emit_tail(pending)
                    pending = (t, b, small_ps, stats_tiles, y_sbs)
            emit_tail(pending)

    nc.compile()
    return nc


def _prep_host_inputs(x, conv_w, conv_b, gamma, beta):
    x = np.asarray(x, dtype=np.float32)
    conv_w = np.asarray(conv_w, dtype=np.float32)
    conv_b = np.asarray(conv_b, dtype=np.float32)
    gamma = np.asarray(gamma, dtype=np.float32)
    beta = np.asarray(beta, dtype=np.float32)

    # lhsT tiles: [ci, prec, k, ci_t, co_t, co]
    Wt = conv_w.transpose(1, 0, 2)                      # [ci_g, co_g, k]
    W6 = Wt.reshape(2, 128, CT, 128, K)                 # [ci_t, ci, co_t, co, k]
    whi32 = W6.astype(ml_dtypes.bfloat16).astype(np.float32)
    wlo = (W6 - whi32).astype(ml_dtypes.bfloat16)
    whi = W6.astype(ml_dtypes.bfloat16)
    w_host = np.stack(
        [whi.transpose(1, 4, 0, 2, 3), wlo.transpose(1, 4, 0, 2, 3)], axis=1
    )                                                   # [ci, prec, k, ci_t, co_t, co]
    w_host = np.ascontiguousarray(w_host)

    b = conv_b
    fields = np.stack(
        [b, gamma, beta, np.float32(L) * b, np.float32(2.0) * b,
         np.float32(L) * b * b]
    )                                                   # [6, 256]
    chan = np.ascontiguousarray(fields.reshape(6, CT, 128).transpose(2, 0, 1))

    onesg = np.zeros((128, 4), np.float32)
    for ci in range(128):
        onesg[ci, ci // GPC] = 1.0
    onesb = np.zeros((128, 128), np.float32)
    for co in range(128):
        onesb[co // GPC, co] = 1.0

    shards = []
    for i in range(N_CORES):
        shards.append(
            {
                "x": np.ascontiguousarray(x[:, i * B_LOC : (i + 1) * B_LOC]),
                "w": w_host,
                "chan": chan,
                "onesg": onesg,
                "onesb": onesb,
            }
        )
    return shards


def kernel(x, conv_w, conv_b, gamma, beta, _trace=False):
    from concourse.bass_utils import run_bass_kernel_spmd

    if "nc" not in _COMPILED:
        _COMPILED["nc"] = _build_program()
    nc = _COMPILED["nc"]

    in_maps = _prep_host_inputs(x, conv_w, conv_b, gamma, beta)
    res = run_bass_kernel_spmd(
        nc, in_maps, list(range(N_CORES)), trace=_trace
    )
    out = np.concatenate([r["y"] for r in res.results], axis=1)
    _COMPILED["last_result"] = res
    return out
